# revision 1
# baseline (speedup 1.0000x reference)
"""EWMA predictor (sliding-window variance, exponentially weighted sum) on 8 trn2 cores.

Math: for j in [0, L): window_j = x[j : j+128], weight ff^(L-1-j),
result = norm * sum_j ff^(L-1-j) * var(window_j, ddof=1),
norm = (1-ff)/(1-ff^L), ff = sigmoid(raw_forgetting_factor).

Sharding: windows split over 8 cores x 128 partitions; partition p of core c
owns the 512 consecutive windows starting at base_c + 512*p and loads the 639
input elements covering them (halo overlap, contiguous per partition). The
per-core input tile carries ff and the per-partition combine coefficients
c_p = ff^i0(c,p)/127 in two extra trailing columns, so the input DMAs load
everything; the input DMA is split into column halves across the SP and ACT
HWDGE rings (a single full-width descriptor set measured ~10x slower).

Per-core device program (vector + scalar + PE engines):
  s1[t], s2[t]: sliding 128-window sums of x and x^2 via tensor_tensor_scan
                recurrence s[t] = (x[t+127] + s[t-1]) - x[t-1]
  d[t] = s2 - s1^2/128 = 127 * var
  e[t] = ff*e[t-1] + d[t]  (scan, ff read via stride-0 broadcast AP)
  contrib[p] = c_p * e[511]; PE matmul against const ones reduces over
  partitions to a single fp32 scalar, copied PSUM->SBUF and DMA'd out as a
  4-byte single-descriptor write (a [128,1] out = 128 descriptors measured
  ~6.4us vs ~free for 1 descriptor).
Host sums the 8 core scalars and applies norm in float64.

build_nc(reps=N) unrolls the body N times with serialized iterations — used
only for wall-clock loop timing (see bench_loop.py); the product kernel uses
reps=1.
"""

import numpy as np

import concourse.bass as bass
import concourse.mybir as mybir
from concourse.bass_utils import run_bass_kernel_spmd

L = 524288          # look-back windows
W = 128             # variance window length
N = L + W           # input length
NCORES = 8
WIN_PER_CORE = L // NCORES      # 65536
RUN = WIN_PER_CORE // 128       # 512 windows per partition
COLS = RUN + W - 1              # 639 input elems per partition
XTW = COLS + 2                  # + ff column + coeff column

_NC_CACHE = {}


def plan_run(ff64: float) -> int:
    """Windows-per-partition for the adaptive program.

    Weights ff^i are EXACTLY zero in fp32 (past subnormals) once
    i > 104/|ln ff|, so the reference's own terms there are zeros and windows
    beyond that cannot affect any output bit. Keep a >=1024-window margin,
    round the 1024*run window count up to a power-of-two run, clamp to
    [8, 512]; run=512 is the exact full computation (all L windows).
    """
    lnff = np.log(np.float64(ff64))
    if not (lnff < -1e-9):
        return RUN
    k_needed = 104.0 / (-lnff)
    run_min = int(np.ceil((k_needed + 1024.0) / 1024.0))
    run = 8
    while run < run_min:
        run *= 2
    return min(run, RUN)


def build_nc(reps: int = 1, run: int = RUN, small: bool | None = None) -> bass.Bass:
    """Per-core program. run=windows/partition. For small runs (<=64) the
    whole chain lives on the DVE (op bodies are tiny, so cross-engine
    semaphore hops cost more than the ACT offload saves, and with no
    activations at all the act-table load disappears); for large runs the
    squares run on the ACT engine overlapping the DVE scans."""
    cols = run + W - 1
    xtw = cols + 2
    if small is None:
        # The cost model favors the ACT-overlapped split chain at every run
        # size (5729 vs 5999 ns at run=8); the all-DVE path is kept for
        # experiments only.
        small = False
    nc = bass.Bass(trn_type="TRN2")
    f32 = mybir.dt.float32
    A = mybir.AluOpType
    xt = nc.declare_dram_parameter("xt", [128, xtw], f32, isOutput=False)
    acc = nc.declare_dram_parameter("acc", [1, 1], f32, isOutput=True)
    HALF = xtw // 2

    with (
        nc.sbuf_tensor([128, xtw], f32) as XT,
        nc.sbuf_tensor([128, cols], f32) as X2,
        nc.sbuf_tensor([128, run], f32) as S1,
        nc.sbuf_tensor([128, run], f32) as S2,
        nc.sbuf_tensor([128, run], f32) as T2,
        nc.sbuf_tensor([128, run], f32) as D,
        nc.sbuf_tensor([128, run], f32) as E,
        nc.sbuf_tensor([1, 1], f32) as SB11,
        nc.sbuf_tensor([128, 1], f32) as WU2,
        nc.psum_tensor([1, 1], f32) as P11,
        nc.semaphore() as dsem,
        nc.semaphore() as vsem,
        nc.semaphore() as ssem,
        nc.semaphore() as psem,
        nc.Block() as block,
    ):
        NV = 8 if small else 6  # vsem increments per iteration

        @block.sync
        def _(sync):
            for r in range(reps):
                sync.dma_start(XT[:, 0:HALF], xt[:, 0:HALF]).then_inc(dsem, 16)
                sync.wait_ge(dsem, 48 * r + 48)

        @block.scalar
        def _(scalar):
            for r in range(reps):
                if r > 0:
                    scalar.wait_ge(dsem, 48 * r)  # prior iter fully done
                # second input half on the ACT HWDGE ring, parallel with SP
                scalar.dma_start(XT[:, HALF:xtw], xt[:, HALF:xtw]).then_inc(dsem, 16)
                if small:
                    # PSUM -> SBUF -> DRAM, all on ACT (fewer cross-engine hops)
                    scalar.wait_ge(psem, r + 1)
                    scalar.copy(SB11[:], P11[:]).then_inc(ssem, 1)
                    scalar.wait_ge(ssem, r + 1)  # RAW: out-DMA reads SB11
                    scalar.dma_start(acc[:], SB11[:]).then_inc(dsem, 16)
                    continue
                if r == 0:
                    # warmup: pull the activation-table load off the critical
                    # path (runs during the input DMA; result never read)
                    scalar.square(WU2[:], nc.const_aps.tensor(0.0, (128, 1)))
                scalar.wait_ge(dsem, 48 * r + 32)
                scalar.square(X2[:], XT[:, 0:cols]).then_inc(ssem, 1)
                scalar.wait_ge(vsem, NV * r + 2)
                scalar.square(T2[:], S1[:]).then_inc(ssem, 1)
                scalar.wait_ge(psem, r + 1)
                scalar.copy(SB11[:], P11[:]).then_inc(ssem, 1)
                scalar.wait_ge(ssem, 3 * r + 3)  # RAW: out-DMA reads SB11
                scalar.dma_start(acc[:], SB11[:]).then_inc(dsem, 16)

        @block.vector
        def _(vector):
            for r in range(reps):
                vector.wait_ge(dsem, 48 * r + 32)
                if small:
                    vector.scalar_tensor_tensor(
                        X2[:], XT[:, 0:cols], 1.0, XT[:, 0:cols],
                        op0=A.mult, op1=A.mult,
                    ).then_inc(vsem, 1)  # 1
                vector.reduce_sum(
                    S1[:, 0:1], XT[:, 0:W], axis=mybir.AxisListType.X
                ).then_inc(vsem, 1)
                vector.wait_ge(vsem, NV * r + (2 if small else 1))  # RAW: initial
                vector.tensor_tensor_scan(
                    S1[:, 1:run], XT[:, W:cols], XT[:, 0 : run - 1],
                    initial=S1[:, 0:1], op0=A.add, op1=A.subtract,
                ).then_inc(vsem, 1)
                if not small:
                    vector.wait_ge(ssem, 3 * r + 1)  # X2 from ACT
                vector.reduce_sum(
                    S2[:, 0:1], X2[:, 0:W], axis=mybir.AxisListType.X
                ).then_inc(vsem, 1)
                vector.wait_ge(vsem, NV * r + (4 if small else 3))  # RAW: initial
                vector.tensor_tensor_scan(
                    S2[:, 1:run], X2[:, W:cols], X2[:, 0 : run - 1],
                    initial=S2[:, 0:1], op0=A.add, op1=A.subtract,
                ).then_inc(vsem, 1)
                if small:
                    vector.wait_ge(vsem, NV * r + 5)  # RAW: T2 reads S1/S2 path
                    vector.scalar_tensor_tensor(
                        T2[:], S1[:], 1.0, S1[:], op0=A.mult, op1=A.mult
                    ).then_inc(vsem, 1)  # 6
                    vector.wait_ge(vsem, NV * r + 6)
                else:
                    vector.wait_ge(ssem, 3 * r + 2)  # T2 from ACT
                    vector.wait_ge(vsem, NV * r + 4)  # RAW: D reads S2
                vector.scalar_tensor_tensor(
                    D[:], T2[:], -1.0 / 128.0, S2[:], op0=A.mult, op1=A.add
                ).then_inc(vsem, 1)
                vector.wait_ge(vsem, NV * r + (7 if small else 5))  # RAW: E reads D
                vector.tensor_tensor_scan(
                    E[:], XT[:, cols : cols + 1].broadcast_to([128, run]), D[:],
                    initial=0.0, op0=A.mult, op1=A.add,
                ).then_inc(vsem, 1)  # small: 8, big: 6

        @block.tensor
        def _(tensor):
            for r in range(reps):
                tensor.wait_ge(vsem, NV * r + (8 if small else 6))
                # weighted cross-partition reduce: sum_p E_last[p] * c_p
                tensor.matmul(
                    P11[:], E[:, run - 1 : run], XT[:, cols + 1 : cols + 2]
                ).then_inc(psem, 1)

    return nc


def _get_nc(run: int = RUN) -> bass.Bass:
    if run not in _NC_CACHE:
        _NC_CACHE[run] = build_nc(run=run)
    return _NC_CACHE[run]


def make_in_maps(
    x: np.ndarray, ff32: np.float32, run: int = RUN
) -> list[dict[str, np.ndarray]]:
    """Per-core input tiles covering the last 1024*run windows (all L windows
    when run=512); slot (c, p) owns windows starting at
    L - 1024*run + (c*128 + p)*run."""
    cols = run + W - 1
    start0 = L - 1024 * run
    lnff = np.log(np.float64(ff32))
    p = np.arange(128)
    in_maps = []
    for c in range(NCORES):
        base = start0 + c * 128 * run
        xt = np.empty((128, cols + 2), dtype=np.float32)
        xt[:, 0:cols] = np.lib.stride_tricks.as_strided(
            x[base:], shape=(128, cols), strides=(run * 4, 4)
        )
        xt[:, cols] = ff32
        # combine coefficient: weight of this partition's last window / 127
        i0 = L - 1 - (base + run * p + (run - 1))
        xt[:, cols + 1] = (np.exp(lnff * i0) / 127.0).astype(np.float32)
        in_maps.append({"xt": xt})
    return in_maps


def combine_host(accs: list[np.ndarray], ff32: np.float32) -> np.ndarray:
    """accs: per-core [1,1] device partial sums. Float64 host reduction."""
    ff64 = np.float64(ff32)
    total = np.float64(0.0)
    for c in range(NCORES):
        total += np.float64(np.asarray(accs[c]).reshape(()))
    norm = (1.0 - ff64) / (1.0 - np.exp(np.log(ff64) * L))
    return np.asarray(np.float32(norm * total))


def kernel(past_returns, features, raw_forgetting_factor):
    x = np.ascontiguousarray(np.asarray(past_returns, dtype=np.float32))
    assert x.shape == (N,), x.shape
    raw = np.float64(np.asarray(raw_forgetting_factor).reshape(-1)[0])
    ff32 = np.float32(1.0 / (1.0 + np.exp(-raw)))

    run = plan_run(np.float64(ff32))
    nc = _get_nc(run)
    in_maps = make_in_maps(x, ff32, run)
    res = run_bass_kernel_spmd(nc, in_maps, list(range(NCORES)))
    accs = [res.results[c]["acc"] for c in range(NCORES)]
    return combine_host(accs, ff32)



# revision 8
# speedup vs baseline: 5.6443x; 5.6443x over previous
"""EWMA predictor (sliding-window variance, exponentially weighted sum) on 8 trn2 cores.

Math: for age i in [0, L): window_i = x[N-1-W-i : N-1-i], weight ff^i,
result = norm * sum_i ff^i * var(window_i, ddof=1),
norm = (1-ff)/(1-ff^L), ff = sigmoid(raw_forgetting_factor).

Weights decay geometrically, so only the newest K windows contribute: the
dropped tail is bounded by ff^K (relative). With the checked-in parameter
(raw=3.4, ff=0.9677) ff^1024 ~ 2e-15, far below the fp32 reference's own
noise floor; the reference's fp32 ff**i literally underflows to 0 beyond
i~3100. The fast path therefore computes the newest K=1024 windows exactly
(in fp32 stats over fp16 samples) and falls back to the previous full
kernel when ff is close enough to 1 that the tail could matter
(ff^1024 > 1e-6).

Fast path, per core (128 windows, one per partition):
  - GPSIMD: iota (int16 token idxs, standard lib) -> load attnmlp lib ->
    dma_gather pulls 128 windows x 128 fp16 samples from HBM straight into
    SBUF (one 256B token per window).
  - DVE: bn_stats + bn_aggr compute each partition's mean/var in fp32.
  - GPSIMD: kv_writeback ships the [128,1] variance column back to HBM.
  Host applies the exponential weights / (1-ff) normalization in float64 and
  the ddof-1 correction (x128/127).
  The gather/writeback path avoids the HWDGE descriptor-generation floor and
  DMA completion latency that dominate a conventional load->compute->store
  kernel of this size.

Fallback path (ff ~ 1): the original kernel - windows split over
8 cores x 128 partitions, sliding sums via tensor_tensor_scan, exponential
scan, PE matmul reduce; computes all L windows exactly at run=512.
"""

import numpy as np

import bass_rust as _bass_rust
import concourse.bass as bass
import concourse.mybir as mybir
from concourse.bass_utils import run_bass_kernel_spmd
from concourse.library_config import all_libraries, standard as _std_lib

L = 524288          # look-back windows
W = 128             # variance window length
N = L + W           # input length
NCORES = 8
KWIN = 1024         # fast-path window count (128 per core)
WIN_PER_CORE = L // NCORES      # 65536
RUN = WIN_PER_CORE // 128       # 512 windows per partition
COLS = RUN + W - 1              # 639 input elems per partition
XTW = COLS + 2                  # + ff column + coeff column

_NC_CACHE = {}


# ---------------------------------------------------------------------------
# Fast path
# ---------------------------------------------------------------------------

def build_fast_nc() -> bass.Bass:
    f32 = mybir.dt.float32
    f16 = mybir.dt.float16
    i16 = mybir.dt.int16
    i32 = mybir.dt.int32

    nc = bass.Bass(trn_type="TRN2")
    # rows 0..127: window per row; rows 128..255: zero padding (the gather
    # idx tensor spans 128 partitions with values up to 239, only the first
    # 16 partitions / 128 tokens are consumed).
    xt = nc.declare_dram_parameter("xt", [256, 128], f16, isOutput=False)
    acc = nc.declare_dram_parameter("acc", [1, 128, 1, 1], f32, isOutput=True)

    with (
        nc.sbuf_tensor([128, 1, 128], f16) as XT,
        nc.sbuf_tensor([128, 6], f32) as ST,
        nc.sbuf_tensor([128, 1, 1, 2], f32) as OUT,
        nc.sbuf_tensor([128, 1], i32) as IDX32,
        nc.sbuf_tensor([128, 8], i16) as IDX16,
        nc.semaphore() as gsem,
        nc.semaphore() as vsem,
        nc.semaphore() as psem,
        nc.semaphore() as wsem,
        nc.Block() as block,
    ):
        @block.vector
        def _(vector):
            vector.memset(IDX32[:, :], 0).then_inc(vsem, 4)
            vector.wait_ge(gsem, 16)
            vector.bn_stats(ST[:, :], XT[:, 0, :]).then_inc(vsem, 1)
            vector.wait_ge(vsem, 5)
            vector.bn_aggr(OUT[:, 0, 0, 0:2], ST[:, :]).then_inc(vsem, 1)

        @block.gpsimd
        def _(gpsimd):
            gpsimd.iota(
                IDX16[:, :], pattern=[[16, 8]], base=0, channel_multiplier=1
            ).then_inc(psem, 1)
            gpsimd.wait_ge(psem, 1)
            gpsimd.dma_gather(
                XT[:, :, :], xt[:, :], IDX16[:, :],
                num_idxs=128, num_idxs_reg=128, elem_size=128,
            ).then_inc(gsem, 16)
            gpsimd.wait_ge(vsem, 6)
            gpsimd.kv_writeback(
                acc[:, :, :, :],
                OUT[:, :, :, 1:2],
                IDX32[:, :],
            ).then_inc(wsem, 16)

    # The GPSIMD custom instructions (dma_gather / kv_writeback) need their
    # ucode library loaded; run the two Bacc compile passes that insert the
    # reload and lower it to concrete ISA (plain Bass.finalize doesn't, and
    # the full Bacc pipeline is incompatible with the bass2jax PJRT path).
    mask: dict[type, int] = {}
    for lib in all_libraries:
        for it in lib.instructions:
            mask[it] = mask.get(it, 0) | (1 << lib.index)
    _bass_rust.insert_library_loads(nc, mask, len(all_libraries), _std_lib.index)
    mybir.codegen_inst_isa_subclasses(nc)
    return nc


def make_in_maps_fast(x: np.ndarray) -> list[dict[str, np.ndarray]]:
    """Core c, partition p owns window age i=128c+p: x[N-1-W-i : N-1-i]."""
    x16 = x.astype(np.float16)
    base = np.lib.stride_tricks.sliding_window_view(x16, W)  # [N-W+1, W]
    in_maps = []
    for c in range(NCORES):
        xt = np.zeros((256, 128), dtype=np.float16)
        # age i window starts at N-1-W-i; ages c*128 .. c*128+127
        starts = N - 1 - W - (128 * c + np.arange(128))
        xt[0:128, :] = base[starts, :]
        in_maps.append({"xt": xt})
    return in_maps


def combine_host_fast(accs: list[np.ndarray], ff64: float) -> np.ndarray:
    lnff = np.log(np.float64(ff64))
    total = np.float64(0.0)
    for c in range(NCORES):
        var_b = np.asarray(accs[c], dtype=np.float64).reshape(128)
        i = 128 * c + np.arange(128)
        total += np.sum(np.exp(lnff * i) * var_b)
    # biased -> ddof=1, then EWMA normalization
    total *= np.float64(W) / np.float64(W - 1)
    norm = (1.0 - ff64) / (1.0 - np.exp(lnff * L))
    return np.asarray(np.float32(norm * total))


def fast_path_ok(ff64: float) -> bool:
    """Tail beyond KWIN windows is bounded by ff^KWIN relative; require
    < 1e-6 (tolerance is 2e-2)."""
    lnff = np.log(np.float64(ff64))
    return bool(lnff * KWIN < np.log(1e-6))


# ---------------------------------------------------------------------------
# Fallback path (original kernel)
# ---------------------------------------------------------------------------

def plan_run(ff64: float) -> int:
    """Windows-per-partition for the fallback program. Weights ff^i are
    EXACTLY zero in fp32 (past subnormals) once i > 104/|ln ff|; keep a
    >=1024-window margin, round up to a power-of-two run in [8, 512]."""
    lnff = np.log(np.float64(ff64))
    if not (lnff < -1e-9):
        return RUN
    k_needed = 104.0 / (-lnff)
    run_min = int(np.ceil((k_needed + 1024.0) / 1024.0))
    run = 8
    while run < run_min:
        run *= 2
    return min(run, RUN)


def build_nc(reps: int = 1, run: int = RUN, small: bool | None = None) -> bass.Bass:
    """Original per-core program (see module docstring); kept as the exact
    fallback for ff ~ 1."""
    cols = run + W - 1
    xtw = cols + 2
    if small is None:
        small = False
    nc = bass.Bass(trn_type="TRN2")
    f32 = mybir.dt.float32
    A = mybir.AluOpType
    xt = nc.declare_dram_parameter("xt", [128, xtw], f32, isOutput=False)
    acc = nc.declare_dram_parameter("acc", [1, 1], f32, isOutput=True)
    HALF = xtw // 2

    with (
        nc.sbuf_tensor([128, xtw], f32) as XT,
        nc.sbuf_tensor([128, cols], f32) as X2,
        nc.sbuf_tensor([128, run], f32) as S1,
        nc.sbuf_tensor([128, run], f32) as S2,
        nc.sbuf_tensor([128, run], f32) as T2,
        nc.sbuf_tensor([128, run], f32) as D,
        nc.sbuf_tensor([128, run], f32) as E,
        nc.sbuf_tensor([1, 1], f32) as SB11,
        nc.sbuf_tensor([128, 1], f32) as WU2,
        nc.psum_tensor([1, 1], f32) as P11,
        nc.semaphore() as dsem,
        nc.semaphore() as vsem,
        nc.semaphore() as ssem,
        nc.semaphore() as psem,
        nc.Block() as block,
    ):
        NV = 8 if small else 6  # vsem increments per iteration

        @block.sync
        def _(sync):
            for r in range(reps):
                sync.dma_start(XT[:, 0:HALF], xt[:, 0:HALF]).then_inc(dsem, 16)
                sync.wait_ge(dsem, 48 * r + 48)

        @block.scalar
        def _(scalar):
            for r in range(reps):
                if r > 0:
                    scalar.wait_ge(dsem, 48 * r)  # prior iter fully done
                scalar.dma_start(XT[:, HALF:xtw], xt[:, HALF:xtw]).then_inc(dsem, 16)
                if small:
                    scalar.wait_ge(psem, r + 1)
                    scalar.copy(SB11[:], P11[:]).then_inc(ssem, 1)
                    scalar.wait_ge(ssem, r + 1)
                    scalar.dma_start(acc[:], SB11[:]).then_inc(dsem, 16)
                    continue
                if r == 0:
                    scalar.square(WU2[:], nc.const_aps.tensor(0.0, (128, 1)))
                scalar.wait_ge(dsem, 48 * r + 32)
                scalar.square(X2[:], XT[:, 0:cols]).then_inc(ssem, 1)
                scalar.wait_ge(vsem, NV * r + 2)
                scalar.square(T2[:], S1[:]).then_inc(ssem, 1)
                scalar.wait_ge(psem, r + 1)
                scalar.copy(SB11[:], P11[:]).then_inc(ssem, 1)
                scalar.wait_ge(ssem, 3 * r + 3)  # RAW: out-DMA reads SB11
                scalar.dma_start(acc[:], SB11[:]).then_inc(dsem, 16)

        @block.vector
        def _(vector):
            for r in range(reps):
                vector.wait_ge(dsem, 48 * r + 32)
                if small:
                    vector.scalar_tensor_tensor(
                        X2[:], XT[:, 0:cols], 1.0, XT[:, 0:cols],
                        op0=A.mult, op1=A.mult,
                    ).then_inc(vsem, 1)
                vector.reduce_sum(
                    S1[:, 0:1], XT[:, 0:W], axis=mybir.AxisListType.X
                ).then_inc(vsem, 1)
                vector.wait_ge(vsem, NV * r + (2 if small else 1))
                vector.tensor_tensor_scan(
                    S1[:, 1:run], XT[:, W:cols], XT[:, 0 : run - 1],
                    initial=S1[:, 0:1], op0=A.add, op1=A.subtract,
                ).then_inc(vsem, 1)
                if not small:
                    vector.wait_ge(ssem, 3 * r + 1)  # X2 from ACT
                vector.reduce_sum(
                    S2[:, 0:1], X2[:, 0:W], axis=mybir.AxisListType.X
                ).then_inc(vsem, 1)
                vector.wait_ge(vsem, NV * r + (4 if small else 3))
                vector.tensor_tensor_scan(
                    S2[:, 1:run], X2[:, W:cols], X2[:, 0 : run - 1],
                    initial=S2[:, 0:1], op0=A.add, op1=A.subtract,
                ).then_inc(vsem, 1)
                if small:
                    vector.wait_ge(vsem, NV * r + 5)
                    vector.scalar_tensor_tensor(
                        T2[:], S1[:], 1.0, S1[:], op0=A.mult, op1=A.mult
                    ).then_inc(vsem, 1)
                    vector.wait_ge(vsem, NV * r + 6)
                else:
                    vector.wait_ge(ssem, 3 * r + 2)  # T2 from ACT
                    vector.wait_ge(vsem, NV * r + 4)  # RAW: D reads S2
                vector.scalar_tensor_tensor(
                    D[:], T2[:], -1.0 / 128.0, S2[:], op0=A.mult, op1=A.add
                ).then_inc(vsem, 1)
                vector.wait_ge(vsem, NV * r + (7 if small else 5))
                vector.tensor_tensor_scan(
                    E[:], XT[:, cols : cols + 1].broadcast_to([128, run]), D[:],
                    initial=0.0, op0=A.mult, op1=A.add,
                ).then_inc(vsem, 1)

        @block.tensor
        def _(tensor):
            for r in range(reps):
                tensor.wait_ge(vsem, NV * r + (8 if small else 6))
                tensor.matmul(
                    P11[:], E[:, run - 1 : run], XT[:, cols + 1 : cols + 2]
                ).then_inc(psem, 1)

    return nc


def _get_nc(run: int = RUN) -> bass.Bass:
    key = ("slow", run)
    if key not in _NC_CACHE:
        _NC_CACHE[key] = build_nc(run=run)
    return _NC_CACHE[key]


def _get_fast_nc() -> bass.Bass:
    if "fast" not in _NC_CACHE:
        _NC_CACHE["fast"] = build_fast_nc()
    return _NC_CACHE["fast"]


def make_in_maps(
    x: np.ndarray, ff32: np.float32, run: int = RUN
) -> list[dict[str, np.ndarray]]:
    """Fallback-path input tiles covering the last 1024*run windows."""
    cols = run + W - 1
    start0 = L - 1024 * run
    lnff = np.log(np.float64(ff32))
    p = np.arange(128)
    in_maps = []
    for c in range(NCORES):
        base = start0 + c * 128 * run
        xt = np.empty((128, cols + 2), dtype=np.float32)
        xt[:, 0:cols] = np.lib.stride_tricks.as_strided(
            x[base:], shape=(128, cols), strides=(run * 4, 4)
        )
        xt[:, cols] = ff32
        i0 = L - 1 - (base + run * p + (run - 1))
        xt[:, cols + 1] = (np.exp(lnff * i0) / 127.0).astype(np.float32)
        in_maps.append({"xt": xt})
    return in_maps


def combine_host(accs: list[np.ndarray], ff32: np.float32) -> np.ndarray:
    ff64 = np.float64(ff32)
    total = np.float64(0.0)
    for c in range(NCORES):
        total += np.float64(np.asarray(accs[c]).reshape(()))
    norm = (1.0 - ff64) / (1.0 - np.exp(np.log(ff64) * L))
    return np.asarray(np.float32(norm * total))


def kernel(past_returns, features, raw_forgetting_factor):
    x = np.ascontiguousarray(np.asarray(past_returns, dtype=np.float32))
    assert x.shape == (N,), x.shape
    raw = np.float64(np.asarray(raw_forgetting_factor).reshape(-1)[0])
    ff32 = np.float32(1.0 / (1.0 + np.exp(-raw)))
    ff64 = np.float64(ff32)

    if fast_path_ok(ff64):
        nc = _get_fast_nc()
        in_maps = make_in_maps_fast(x)
        res = run_bass_kernel_spmd(nc, in_maps, list(range(NCORES)))
        accs = [res.results[c]["acc"] for c in range(NCORES)]
        return combine_host_fast(accs, ff64)

    run = plan_run(ff64)
    nc = _get_nc(run)
    in_maps = make_in_maps(x, ff32, run)
    res = run_bass_kernel_spmd(nc, in_maps, list(range(NCORES)))
    accs = [res.results[c]["acc"] for c in range(NCORES)]
    return combine_host(accs, ff32)
